# revision 36
# baseline (speedup 1.0000x reference)
"""DispMVS depth-fusion kernel for 8 Trainium2 NeuronCores.

Sharding: core c handles batch b = c // 4 and coarse rows r0 = (c % 4) * 64
(64 of 256 rows), with BOTH neighbor streams (NN=2) resident on the core
(partitions = nn*64 + row).  The cross-neighbor confidence-fusion softmax is
then core-local; cores never communicate.

Pipeline per core (one Bass/Tile program, identical for all 8 cores):
  1. geometry: elementwise epipolar math -> clipped inverse depth  [128, 330]
     (330 = 320 cols + 10 packed halo pixels/partition for the rows just
     outside the chunk, needed by the 3x3 unfold).
  2. DRAM scratch round-trip to rebuild inv-depth as 3 vertically shifted,
     zero-padded rows per partition (the unfold operand); conf comes the same
     way directly from a host-padded input.
  3. per (p, w-half) chunk: exp(mask) on ACT, grouped 9-way reductions on DVE
     (softmax numerators/denominator), convex-upsample of inv-depth and conf,
     then the 2-view softmax fusion and final reciprocal.
"""

import numpy as np

NN, B, H, W = 2, 2, 256, 320
UP = 4
EPS = 1e-6
RPC = 64          # coarse rows per core
NCORES = 8
HW = H * W
RW = RPC * W      # elements in one [64, 320] channel-slice

# consts columns
(
    C_M00, C_M01, C_M02, C_M10, C_M11, C_M12, C_M20, C_M21, C_M22,
    C_T0, C_T1, C_T2,
    C_R00, C_R01, C_R02, C_R10, C_R11, C_R12, C_R20, C_R21, C_R22,
    C_A0, C_A1, C_A2, C_B0, C_B1, C_B2,
    C_TX, C_TY, C_TZ,
    C_CA, C_CB, C_DS, C_DB, C_TEN,
) = range(35)
NCONST = 36

_cache = {}


def _register_custom_ops():
    """Register this kernel's custom DVE ops (idempotent). Returns a dict.

    MUL_CUMSUM_ANT: out = cumsum(in0*in1) along the free stream - grouped
      9-tap weighted sums fall out as differences of every-9th prefix value,
      one line-rate pass instead of multiply + strided TENSOR_REDUCE passes.
    SUMSQ_ANT: out = in0^2 + in1^2 (one pass instead of 3).
    RSQRT_NR_ANT: one Newton step for 1/sqrt: out = in0*(1.5 - 0.5*in1*in0^2)
      (one pass instead of 4).
    """
    from concourse import dve_ops
    from concourse.dve_spec import AluOp, C0, C1, Spec, Src0, Src1, _has_src1, lower, scan
    from concourse.dve_uop import DveOpSpec

    have = {o.name: o for o in dve_ops.OPS}
    if "MUL_CUMSUM_ANT" in have:
        return have

    def cum_ref(in0, in1, s0, s1, imm2):
        a = in0.astype(np.float32).reshape(in0.shape[0], -1) * in1.astype(
            np.float32
        ).reshape(in1.shape[0], -1)
        return np.cumsum(a, axis=1, dtype=np.float32).reshape(in0.shape)

    specs = [
        ("MUL_CUMSUM_ANT", Spec(body=scan(AluOp.ADD, Src0 * Src1), reference=cum_ref)),
        (
            "SUMSQ_ANT",
            Spec(
                body=Src0 * Src0 + Src1 * Src1,
                reference=lambda in0, in1, s0, s1, imm2: (
                    in0.astype(np.float32) ** 2 + in1.astype(np.float32) ** 2
                ),
            ),
        ),
        (
            "RSQRT_NR_ANT",
            Spec(
                body=(Src0 * Src0 * Src1 * C0 + C1) * Src0,
                reference=lambda in0, in1, s0, s1, imm2: (
                    (in0.astype(np.float32) ** 2 * in1 * s0 + s1) * in0
                ),
            ),
        ),
    ]
    out = {}
    for name, spec in specs:
        op = dve_ops.DveOp(name, spec, subdim=False, uops_sha={})
        dve_ops.OPS.append(op)
        dve_ops.CUSTOM_DVE_SPECS[name] = spec
        dve_ops._SUB_OPCODE_FOR_NAME[name] = (
            dve_ops._CUSTOM_DVE_ROW_BASE + len(dve_ops.OPS) - 1
        )
        for ver in ("v3", "v4"):
            tmp = DveOpSpec(
                name=name,
                opcode=dve_ops.get_dve_sub_opcode(name),
                uops=lower(spec, ver=ver),
                rd1_en=_has_src1(spec),
            )
            op.uops_sha[ver] = tmp.sha(ver)
        out[name] = op
    assert max(dve_ops._SUB_OPCODE_FOR_NAME.values()) < 0x20
    return out


def _build_program():
    import concourse.bass as bass
    import concourse.bacc as bacc
    import concourse.tile as tile
    from concourse import mybir
    from concourse.alu_op_type import AluOpType as op

    f32 = mybir.dt.float32
    i32 = mybir.dt.int32
    Act = mybir.ActivationFunctionType

    cops = _register_custom_ops()
    nc = bacc.Bacc("TRN2", target_bir_lowering=False, debug=False)

    pix_d = nc.dram_tensor("pix", [128, 4, 330], f32, kind="ExternalInput").ap()
    consts_d = nc.dram_tensor("consts", [128, NCONST], f32, kind="ExternalInput").ap()
    hm_d = nc.dram_tensor("hm", [128, 10], f32, kind="ExternalInput").ap()
    confpad_d = nc.dram_tensor("confpad", [NN, 66, 322], f32, kind="ExternalInput").ap()
    # mask pre-packed on host to [gc, wc, (nn,r), q, w, k] (k innermost) so each
    # chunk's DMA is one contiguous [128, 5760] transfer and the 9-tap groups
    # are unit-stride for the cumsum trick
    mask_d = nc.dram_tensor("maskpk", [4, 2, 128, 4, 160, 9], f32, kind="ExternalInput").ap()
    scr = nc.dram_tensor("scr", [NN, 66, 322], f32, kind="Internal").ap()
    out_d = nc.dram_tensor("out", [RPC * UP, W * UP], f32, kind="ExternalOutput").ap()

    def dram_ap(base, off, dims):
        return bass.AP(tensor=base.tensor, offset=base.offset + off, ap=[list(d) for d in dims])

    with tile.TileContext(nc) as tc:
        with tc.tile_pool(name="persist", bufs=1) as pp:
            ep_ctx = tc.tile_pool(name="early", bufs=1)
            ep = ep_ctx.__enter__()
            consts = pp.tile([128, NCONST], f32, name="consts")
            nc.sync.dma_start(out=consts[:], in_=consts_d)

            def CC(i, p0=0, p1=128):
                return consts[p0:p1, i : i + 1]

            pix = ep.tile([128, 4, 330], f32, name="pix")
            nc.sync.dma_start(out=pix[:], in_=pix_d)
            hm = ep.tile([128, 10], f32, name="hm")
            nc.sync.dma_start(out=hm[:], in_=hm_d)

            t3i = ep.tile([128, 3, 322], f32, name="t3i")  # unfold rows of inv-depth
            t3c = ep.tile([128, 3, 322], f32, name="t3c")  # unfold rows of conf
            # conf unfold rows straight from the host-padded input
            for nn in range(NN):
                src = dram_ap(
                    confpad_d, nn * 66 * 322,
                    [[322, 64], [322, 3], [1, 322]],
                )
                nc.sync.dma_start(out=t3c[nn * 64 : nn * 64 + 64], in_=src)

            inv_res = ep.tile([128, 330], f32, name="inv_res")
            zero2 = ep.tile([2, 132], f32, name="zero2")
            nc.vector.memset(zero2[:], 0.0)

            # ---------------- geometry ----------------
            u = pix[:, 0, :]
            v = pix[:, 1, :]
            d = pix[:, 2, :]
            fl = pix[:, 3, :]

            with tc.tile_pool(name="geom", bufs=1) as gp:
                _tagn = [0]

                def T(shape=(128, 330)):
                    _tagn[0] += 1
                    return gp.tile(list(shape), f32, name=f"g{_tagn[0]}", tag=f"g{_tagn[0]}")

                def TT(o, a, b, alu):
                    nc.vector.tensor_tensor(out=o, in0=a, in1=b, op=alu)

                def TS(o, a, s1, o0, s2=None, o1=None):
                    if o1 is None:
                        nc.vector.tensor_scalar(out=o, in0=a, scalar1=s1, scalar2=None, op0=o0)
                    else:
                        nc.vector.tensor_scalar(out=o, in0=a, scalar1=s1, scalar2=s2, op0=o0, op1=o1)

                def STT(o, a, s, b, o0, o1):
                    nc.vector.scalar_tensor_tensor(out=o, in0=a, scalar=s, in1=b, op0=o0, op1=o1)

                def AB(o, a):
                    nc.scalar.activation(out=o, in_=a, func=Act.Abs)

                def AF(o, a, scale, bias):
                    nc.scalar.activation(out=o, in_=a, func=Act.Identity, scale=scale, bias=bias)

                def recip_acc(o, x):
                    t = T()
                    nc.vector.reciprocal_approx_accurate(out=o, in_=x, scratch=t[:])

                # a_j = M @ [u, v, 1]
                a0, a1, a2 = T(), T(), T()
                tmp = T()
                AF(tmp[:], u, CC(C_M00), CC(C_M02))
                STT(a0[:], v, CC(C_M01), tmp[:], op.mult, op.add)
                AF(tmp[:], u, CC(C_M10), CC(C_M12))
                STT(a1[:], v, CC(C_M11), tmp[:], op.mult, op.add)
                AF(tmp[:], u, CC(C_M20), CC(C_M22))
                STT(a2[:], v, CC(C_M21), tmp[:], op.mult, op.add)

                d10 = T()
                AF(d10[:], d, 1.0, CC(C_TEN))

                # z components and their reciprocals
                ps2, pe2, rs2, re2 = T(), T(), T(), T()
                m = T()
                TT(m[:], a2[:], d, op.mult)
                AF(ps2[:], m[:], 1.0, CC(C_T2))
                TT(m[:], a2[:], d10[:], op.mult)
                TT(pe2[:], m[:], ps2[:], op.add)
                AB(m[:], ps2[:])
                TS(m[:], m[:], EPS, op.add)
                recip_acc(rs2[:], m[:])
                AB(m[:], pe2[:])
                TS(m[:], m[:], EPS, op.add)
                recip_acc(re2[:], m[:])

                # x/y components, start and end projections
                pxs, pys, pxe, pye = T(), T(), T(), T()
                for aj, tj, po_s, po_e in ((a0, C_T0, pxs, pxe), (a1, C_T1, pys, pye)):
                    psj, pej = T((128, 330)), T((128, 330))
                    TT(m[:], aj[:], d, op.mult)
                    AF(psj[:], m[:], 1.0, CC(tj))
                    TT(m[:], aj[:], d10[:], op.mult)
                    TT(pej[:], m[:], psj[:], op.add)
                    TT(po_s[:], psj[:], rs2[:], op.mult)
                    TT(po_e[:], pej[:], re2[:], op.mult)

                fdx, fdy = T(), T()
                TT(fdx[:], pxe[:], pxs[:], op.subtract)
                TT(fdy[:], pye[:], pys[:], op.subtract)

                # rsqrt(fdx^2 + fdy^2) via magic seed + 2 fused Newton steps
                q = T()
                nc.vector._custom_dve(cops["SUMSQ_ANT"], out=q[:], in0=fdx[:], in1=fdy[:])
                y = T()
                yi = y[:].bitcast(i32)
                TS(yi, q[:].bitcast(i32), 1, op.arith_shift_right)
                TS(yi, yi, -1, op.bitwise_xor)
                TS(yi, yi, 0x5F3759DF + 1, op.add)
                y2 = T()
                nc.vector._custom_dve(
                    cops["RSQRT_NR_ANT"], out=y2[:], in0=y[:], in1=q[:], s0=-0.5, s1=1.5
                )
                nc.vector._custom_dve(
                    cops["RSQRT_NR_ANT"], out=y[:], in0=y2[:], in1=q[:], s0=-0.5, s1=1.5
                )

                fls = T()
                TT(fls[:], fl, y[:], op.mult)
                mx, my = T(), T()
                TT(m[:], fdx[:], fls[:], op.mult)
                TT(mx[:], m[:], pxs[:], op.add)
                TT(m[:], fdy[:], fls[:], op.mult)
                TT(my[:], m[:], pys[:], op.add)

                fm = T()
                fmi = fm[:].bitcast(i32)
                ax = T()
                AB(ax[:], fdx[:])
                AB(m[:], fdy[:])
                TT(fmi, m[:], ax[:], op.is_gt)

                nx, ny = T(), T()
                AF(tmp[:], mx[:], CC(C_A0), CC(C_A2))
                STT(nx[:], my[:], CC(C_A1), tmp[:], op.mult, op.add)
                AF(tmp[:], mx[:], CC(C_B0), CC(C_B2))
                STT(ny[:], my[:], CC(C_B1), tmp[:], op.mult, op.add)

                rx, ry, rz = T(), T(), T()
                AF(tmp[:], u, CC(C_R00), CC(C_R02))
                STT(rx[:], v, CC(C_R01), tmp[:], op.mult, op.add)
                AF(tmp[:], u, CC(C_R10), CC(C_R12))
                STT(ry[:], v, CC(C_R11), tmp[:], op.mult, op.add)
                AF(tmp[:], u, CC(C_R20), CC(C_R22))
                STT(rz[:], v, CC(C_R21), tmp[:], op.mult, op.add)

                def inv_axis(o, nj, rj, c_t):
                    num = T()
                    TT(m[:], rz[:], nj[:], op.mult)
                    TT(m[:], rj[:], m[:], op.subtract)
                    AB(num[:], m[:])
                    AF(m[:], nj[:], CC(C_TZ), CC(c_t))
                    AB(m[:], m[:])
                    TS(m[:], m[:], EPS, op.add)
                    rden = T()
                    recip_acc(rden[:], m[:])
                    TT(o, num[:], rden[:], op.mult)

                invx, invy = T(), T()
                inv_axis(invx[:], nx, rx, C_TX)
                inv_axis(invy[:], ny, ry, C_TY)

                seld = T()
                nc.vector.select(out=seld[:], mask=fmi, on_true=invy[:], on_false=invx[:])
                AF(tmp[:], seld[:], CC(C_CA), CC(C_CB))
                TS(inv_res[:], tmp[:], 0.0, op.max, 1.0, op.min)

            # zero the halo pixels that fall outside the image (edge chunks)
            nc.vector.tensor_tensor(
                out=inv_res[:, 320:330], in0=inv_res[:, 320:330], in1=hm[:], op=op.mult
            )

            # ------- scratch round-trip: [nn, 66, 322] padded inv-depth -------
            for nn in range(NN):
                base = nn * 66 * 322
                sl = slice(nn * 64, nn * 64 + 64)
                nc.sync.dma_start(
                    out=dram_ap(scr, base + 322 + 1, [[322, 64], [1, 320]]),
                    in_=inv_res[sl, 0:320],
                )
                nc.sync.dma_start(
                    out=dram_ap(scr, base + 1, [[5, 64], [1, 5]]),
                    in_=inv_res[sl, 320:325],
                )
                nc.sync.dma_start(
                    out=dram_ap(scr, base + 65 * 322 + 1, [[5, 64], [1, 5]]),
                    in_=inv_res[sl, 325:330],
                )
                # zero pad columns 0 and 321 of all 66 rows
                nc.sync.dma_start(
                    out=dram_ap(scr, base, [[0, 1], [322, 66], [321, 2]]),
                    in_=zero2[nn : nn + 1, :].rearrange("p (a b) -> p a b", a=66),
                )
            for nn in range(NN):
                src = dram_ap(scr, nn * 66 * 322, [[322, 64], [322, 3], [1, 322]])
                nc.sync.dma_start(out=t3i[nn * 64 : nn * 64 + 64], in_=src)

            # 9 pre-shifted unfold rows per stream (one ACT copy per (dy,dx));
            # then per w-half expanded into exact scan-stream order so the
            # weighted-cumsum reads both ports unit-stride
            uf9i = pp.tile([128, 9, 324], f32, name="uf9i")
            uf9c = pp.tile([128, 9, 324], f32, name="uf9c")
            for t3, uf9 in ((t3i, uf9i), (t3c, uf9c)):
                for dy in range(3):
                    for dx in range(3):
                        nc.scalar.activation(
                            out=uf9[:, dy * 3 + dx, 0 : 322 - dx],
                            in_=t3[:, dy, dx:322],
                            func=Act.Copy,
                        )

            ep_ctx.__exit__(None, None, None)

            # ---------------- upsample + fusion, 2 w-halves x 4 p-chunks ----------------
            WC = 160
            with tc.tile_pool(name="chunk", bufs=2) as cp, tc.tile_pool(
                name="chunk1", bufs=1
            ) as cp1:
                for wc in range(2):
                    w0 = wc * WC
                    # [g(bcast), w, k] expansion of the unfold rows, unit stride
                    ufs = {}
                    for tag, uf9 in (("i", uf9i), ("c", uf9c)):
                        ust = cp1.tile([128, 4, WC, 9], f32, name="ufs" + tag, tag="ufs" + tag)
                        u9b = uf9[:]
                        pdim = list(u9b.ap[0])
                        for g in range(4):
                            nc.gpsimd.tensor_copy(
                                out=ust[:, g],
                                in_=bass.AP(
                                    tensor=u9b.tensor,
                                    offset=u9b.offset + w0,
                                    ap=[pdim, [1, WC], [324, 9]],
                                ),
                            )
                        ufs[tag] = ust
                    for gc in range(4):
                        e = cp.tile([128, 4, WC, 9], f32, name="e", tag="e")
                        nc.sync.dma_start(out=e[:], in_=mask_d[gc, wc])
                        nc.scalar.activation(out=e[:], in_=e[:], func=Act.Exp)

                        # softmax denominator: unit-stride innermost-k reduce
                        s = cp.tile([128, 4, WC], f32, name="s", tag="s")
                        nc.vector.tensor_reduce(
                            out=s[:], in_=e[:], axis=mybir.AxisListType.X, op=op.add
                        )
                        rs = cp.tile([128, 4, WC], f32, name="rs", tag="rs")
                        nc.vector.reciprocal_approx_fast(out=rs[:], in_=s[:])

                        up_t = {}
                        for tag in ("i", "c"):
                            cum = cp1.tile(
                                [128, 4, WC, 9], f32, name="cum", tag="cum"
                            )
                            for g in range(4):
                                nc.vector._custom_dve(
                                    cops["MUL_CUMSUM_ANT"], out=cum[:, g], in0=e[:, g], in1=ufs[tag][:, g]
                                )
                            # every-9th prefix value, with a zero column prepended
                            ce = cp1.tile([128, 4, WC + 1], f32, name="ce", tag="ce" + tag)
                            nc.gpsimd.memset(ce[:, :, 0:1], 0.0)
                            nc.gpsimd.tensor_copy(
                                out=ce[:, :, 1 : WC + 1], in_=cum[:, :, :, 8]
                            )
                            acc = cp.tile([128, 4, WC], f32, name="acc", tag="acc" + tag)
                            nc.vector.tensor_tensor(
                                out=acc[:],
                                in0=ce[:, :, 1 : WC + 1],
                                in1=ce[:, :, 0:WC],
                                op=op.subtract,
                            )
                            upv = cp.tile([128, 4, WC], f32, name="upv", tag="up" + tag)
                            nc.vector.tensor_tensor(out=upv[:], in0=acc[:], in1=rs[:], op=op.mult)
                            up_t[tag] = upv

                        iu, cu = up_t["i"], up_t["c"]
                        lo, hi = slice(0, 64), slice(64, 128)

                        def F(tag):
                            return cp.tile([64, 4, WC], f32, name="f" + tag, tag="f" + tag)

                        # TT operands must share a base partition: move the nn1
                        # halves down to partitions 0-63 via SBUF->SBUF DMA
                        iu2, cu2 = F("iu2"), F("cu2")
                        nc.sync.dma_start(out=iu2[:], in_=iu[hi])
                        nc.sync.dma_start(out=cu2[:], in_=cu[hi])

                        fa, fb, fc, fd = F("a"), F("b"), F("c"), F("d")
                        # fa=dif -> fb=exp(dif) -> fc=1+fb -> fd=1/fc
                        nc.vector.tensor_tensor(out=fa[:], in0=cu2[:], in1=cu[lo], op=op.subtract)
                        nc.scalar.activation(out=fb[:], in_=fa[:], func=Act.Exp)
                        nc.scalar.activation(out=fc[:], in_=fb[:], func=Act.Identity, bias=1.0)
                        nc.vector.reciprocal_approx_fast(out=fd[:], in_=fc[:])
                        # fa=iu1*e -> fc=fa+iu0 -> fa=fc*fd -> fc=scale*fa+bias
                        nc.vector.tensor_tensor(out=fa[:], in0=iu2[:], in1=fb[:], op=op.mult)
                        nc.vector.tensor_tensor(out=fc[:], in0=fa[:], in1=iu[lo], op=op.add)
                        nc.vector.tensor_tensor(out=fa[:], in0=fc[:], in1=fd[:], op=op.mult)
                        nc.scalar.activation(
                            out=fc[:], in_=fa[:], func=Act.Identity,
                            scale=CC(C_DS, 0, 64), bias=CC(C_DB, 0, 64),
                        )
                        out_t = cp.tile([64, WC, 4], f32, name="out_t", tag="out_t")
                        nc.vector.reciprocal_approx_fast(
                            out=out_t[:].rearrange("p w q -> p q w"), in_=fc[:]
                        )
                        dst = dram_ap(
                            out_d,
                            gc * (W * UP) + UP * w0,
                            [[UP * W * UP, 64], [UP, WC], [1, UP]],
                        )
                        nc.sync.dma_start(out=dst, in_=out_t[:])

    nc.finalize()
    return nc


def _host_prep(inputs):
    K_ref = np.asarray(inputs["K_ref"], np.float32)
    K_nei = np.asarray(inputs["K_nei"], np.float32)
    R_nei = np.asarray(inputs["R_nei"], np.float32)
    T_nei = np.asarray(inputs["T_nei"], np.float32)
    depth0 = np.asarray(inputs["depth0"], np.float32)
    flow = np.asarray(inputs["flow"], np.float32)
    mask = np.asarray(inputs["mask"], np.float32)
    conf = np.asarray(inputs["conf"], np.float32)
    dmin = float(np.asarray(inputs["depth_min"]).reshape(-1)[0])
    dmax = float(np.asarray(inputs["depth_max"]).reshape(-1)[0])

    # pixel rays per batch (u, v with unit z)
    uv = []
    for b in range(B):
        Ki = np.linalg.inv(K_ref[b, 0, 0].astype(np.float64))
        gx, gy = np.meshgrid(np.arange(W, dtype=np.float64), np.arange(H, dtype=np.float64))
        x = Ki[0, 0] * gx + Ki[0, 1] * gy + Ki[0, 2]
        y = Ki[1, 0] * gx + Ki[1, 1] * gy + Ki[1, 2]
        z = Ki[2, 0] * gx + Ki[2, 1] * gy + Ki[2, 2]
        uv.append((np.float32(x / z), np.float32(y / z)))

    cA = 1.0 / (dmin - dmax)
    cB = -dmax / (dmin - dmax)

    in_maps = []
    for c in range(NCORES):
        b, rc = c // 4, c % 4
        r0 = rc * RPC
        rtop = max(r0 - 1, 0)
        rbot = min(r0 + RPC, H - 1)

        consts = np.zeros((128, NCONST), np.float32)
        for nn in range(NN):
            Kn = K_nei[nn, b, 0, 0].astype(np.float64)
            Rn = R_nei[nn, b, 0, 0].astype(np.float64)
            Tn = T_nei[nn, b, 0, 0].astype(np.float64).reshape(3)
            M = Kn @ Rn
            t = (Kn @ Tn.reshape(3, 1)).reshape(3)
            iK = np.linalg.inv(Kn)
            assert abs(iK[2, 0]) < 1e-12 and abs(iK[2, 1]) < 1e-12 and abs(iK[2, 2] - 1) < 1e-9
            row = np.zeros(NCONST, np.float32)
            row[C_M00:C_M22 + 1] = M.reshape(-1)
            row[C_T0:C_T2 + 1] = t
            row[C_R00:C_R22 + 1] = Rn.reshape(-1)
            row[C_A0:C_A2 + 1] = iK[0] / (1.0 + EPS)
            row[C_B0:C_B2 + 1] = iK[1] / (1.0 + EPS)
            # C_TX/C_TY feed |tz*n + c| as ACT affine bias -> store negated
            row[C_TX], row[C_TY], row[C_TZ] = -Tn[0], -Tn[1], Tn[2]
            row[C_CA], row[C_CB] = cA, cB
            row[C_TEN] = 10.0
            row[C_DS], row[C_DB] = dmin - dmax, dmax
            consts[nn * 64 : nn * 64 + 64] = row

        u_full, v_full = uv[b]
        d_full = depth0[b, 0]

        pix = np.zeros((128, 4, 330), np.float32)
        for nn in range(NN):
            sl = slice(nn * 64, nn * 64 + 64)
            f_full = flow[nn, b, 0]
            for ch, arr in enumerate((u_full, v_full, d_full, f_full)):
                pix[sl, ch, 0:320] = arr[r0 : r0 + RPC]
                pix[sl, ch, 320:325] = arr[rtop].reshape(64, 5)
                pix[sl, ch, 325:330] = arr[rbot].reshape(64, 5)

        hm = np.ones((128, 10), np.float32)
        if r0 == 0:
            hm[:, 0:5] = 0.0
        if r0 + RPC == H:
            hm[:, 5:10] = 0.0

        confpad = np.zeros((NN, 66, 322), np.float32)
        confpad[:, 1:65, 1:321] = conf[:, b, 0, r0 : r0 + RPC, :]
        if r0 > 0:
            confpad[:, 0, 1:321] = conf[:, b, 0, r0 - 1, :]
        if r0 + RPC < H:
            confpad[:, 65, 1:321] = conf[:, b, 0, r0 + RPC, :]

        # [nn, k, p, q, r, wc, w] -> [p, wc, nn, r, q, w, k]
        ms = mask[:, b, :, r0 : r0 + RPC, :].reshape(NN, 9, 4, 4, RPC, 2, 160)
        mask_pk = np.ascontiguousarray(ms.transpose(2, 5, 0, 4, 3, 6, 1)).reshape(
            4, 2, 128, 4, 160, 9
        )

        in_maps.append(
            {
                "pix": pix,
                "consts": consts,
                "hm": hm,
                "confpad": confpad,
                "maskpk": mask_pk,
            }
        )
    return in_maps


def kernel(**inputs):
    if "nc" not in _cache:
        _cache["nc"] = _build_program()
    nc = _cache["nc"]
    in_maps = _host_prep(inputs)

    from concourse import bass_utils

    res = bass_utils.run_bass_kernel_spmd(nc, in_maps, core_ids=list(range(NCORES)))
    out = np.empty((B, 1, H * UP, W * UP), np.float32)
    for c in range(NCORES):
        b, rc = c // 4, c % 4
        out[b, 0, rc * RPC * UP : (rc + 1) * RPC * UP, :] = res.results[c]["out"]
    return out


# revision 37
# speedup vs baseline: 1.1364x; 1.1364x over previous
"""DispMVS depth-fusion kernel for 8 Trainium2 NeuronCores.

Sharding: core c handles batch b = c // 4 and coarse rows r0 = (c % 4) * 64
(64 of 256 rows), with BOTH neighbor streams (NN=2) resident on the core
(partitions = nn*64 + row).  The cross-neighbor confidence-fusion softmax is
then core-local; cores never communicate.

Pipeline per core (one Bass/Tile program, identical for all 8 cores):
  1. geometry: elementwise epipolar math -> clipped inverse depth  [128, 330]
     (330 = 320 cols + 10 packed halo pixels/partition for the rows just
     outside the chunk, needed by the 3x3 unfold).
  2. DRAM scratch round-trip to rebuild inv-depth as 3 vertically shifted,
     zero-padded rows per partition (the unfold operand); conf comes the same
     way directly from a host-padded input.
  3. per (p, w-half) chunk: exp(mask) on ACT, grouped 9-way reductions on DVE
     (softmax numerators/denominator), convex-upsample of inv-depth and conf,
     then the 2-view softmax fusion and final reciprocal.
"""

import numpy as np

NN, B, H, W = 2, 2, 256, 320
UP = 4
EPS = 1e-6
RPC = 64          # coarse rows per core
NCORES = 8
HW = H * W
RW = RPC * W      # elements in one [64, 320] channel-slice

# consts columns
(
    C_M00, C_M01, C_M02, C_M10, C_M11, C_M12, C_M20, C_M21, C_M22,
    C_T0, C_T1, C_T2,
    C_R00, C_R01, C_R02, C_R10, C_R11, C_R12, C_R20, C_R21, C_R22,
    C_A0, C_A1, C_A2, C_B0, C_B1, C_B2,
    C_TX, C_TY, C_TZ,
    C_CA, C_CB, C_DS, C_DB, C_TEN,
) = range(35)
NCONST = 36

_cache = {}


def _register_custom_ops():
    """Register this kernel's custom DVE ops (idempotent). Returns a dict.

    MUL_CUMSUM_ANT: out = cumsum(in0*in1) along the free stream - grouped
      9-tap weighted sums fall out as differences of every-9th prefix value,
      one line-rate pass instead of multiply + strided TENSOR_REDUCE passes.
    SUMSQ_ANT: out = in0^2 + in1^2 (one pass instead of 3).
    RSQRT_NR_ANT: one Newton step for 1/sqrt: out = in0*(1.5 - 0.5*in1*in0^2)
      (one pass instead of 4).
    """
    from concourse import dve_ops
    from concourse.dve_spec import AluOp, C0, C1, Spec, Src0, Src1, _has_src1, lower, scan
    from concourse.dve_uop import DveOpSpec

    have = {o.name: o for o in dve_ops.OPS}
    if "MUL_CUMSUM_ANT" in have:
        return have

    def cum_ref(in0, in1, s0, s1, imm2):
        a = in0.astype(np.float32).reshape(in0.shape[0], -1) * in1.astype(
            np.float32
        ).reshape(in1.shape[0], -1)
        return np.cumsum(a, axis=1, dtype=np.float32).reshape(in0.shape)

    specs = [
        ("MUL_CUMSUM_ANT", Spec(body=scan(AluOp.ADD, Src0 * Src1), reference=cum_ref)),
        (
            "SUMSQ_ANT",
            Spec(
                body=Src0 * Src0 + Src1 * Src1,
                reference=lambda in0, in1, s0, s1, imm2: (
                    in0.astype(np.float32) ** 2 + in1.astype(np.float32) ** 2
                ),
            ),
        ),
        (
            "RSQRT_NR_ANT",
            Spec(
                body=(Src0 * Src0 * Src1 * C0 + C1) * Src0,
                reference=lambda in0, in1, s0, s1, imm2: (
                    (in0.astype(np.float32) ** 2 * in1 * s0 + s1) * in0
                ),
            ),
        ),
    ]
    out = {}
    for name, spec in specs:
        op = dve_ops.DveOp(name, spec, subdim=False, uops_sha={})
        dve_ops.OPS.append(op)
        dve_ops.CUSTOM_DVE_SPECS[name] = spec
        dve_ops._SUB_OPCODE_FOR_NAME[name] = (
            dve_ops._CUSTOM_DVE_ROW_BASE + len(dve_ops.OPS) - 1
        )
        for ver in ("v3", "v4"):
            tmp = DveOpSpec(
                name=name,
                opcode=dve_ops.get_dve_sub_opcode(name),
                uops=lower(spec, ver=ver),
                rd1_en=_has_src1(spec),
            )
            op.uops_sha[ver] = tmp.sha(ver)
        out[name] = op
    assert max(dve_ops._SUB_OPCODE_FOR_NAME.values()) < 0x20
    return out


def _build_program():
    import concourse.bass as bass
    import concourse.bacc as bacc
    import concourse.tile as tile
    from concourse import mybir
    from concourse.alu_op_type import AluOpType as op

    f32 = mybir.dt.float32
    i32 = mybir.dt.int32
    Act = mybir.ActivationFunctionType

    cops = _register_custom_ops()
    nc = bacc.Bacc("TRN2", target_bir_lowering=False, debug=False)

    pix_d = nc.dram_tensor("pix", [128, 4, 330], f32, kind="ExternalInput").ap()
    consts_d = nc.dram_tensor("consts", [128, NCONST], f32, kind="ExternalInput").ap()
    hm_d = nc.dram_tensor("hm", [128, 10], f32, kind="ExternalInput").ap()
    confpad_d = nc.dram_tensor("confpad", [NN, 66, 322], f32, kind="ExternalInput").ap()
    # mask pre-packed on host to [gc, wc, (nn,r), q, w, k] (k innermost) so each
    # chunk's DMA is one contiguous [128, 5760] transfer and the 9-tap groups
    # are unit-stride for the cumsum trick
    mask_d = nc.dram_tensor("maskpk", [4, 2, 128, 4, 160, 9], f32, kind="ExternalInput").ap()
    scr = nc.dram_tensor("scr", [NN, 66, 322], f32, kind="Internal").ap()
    out_d = nc.dram_tensor("out", [RPC * UP, W * UP], f32, kind="ExternalOutput").ap()

    def dram_ap(base, off, dims):
        return bass.AP(tensor=base.tensor, offset=base.offset + off, ap=[list(d) for d in dims])

    with tile.TileContext(nc) as tc:
        with tc.tile_pool(name="persist", bufs=1) as pp:
            ep_ctx = tc.tile_pool(name="early", bufs=1)
            ep = ep_ctx.__enter__()
            consts = pp.tile([128, NCONST], f32, name="consts")
            nc.sync.dma_start(out=consts[:], in_=consts_d)

            def CC(i, p0=0, p1=128):
                return consts[p0:p1, i : i + 1]

            pix = ep.tile([128, 4, 330], f32, name="pix")
            nc.sync.dma_start(out=pix[:], in_=pix_d)
            hm = ep.tile([128, 10], f32, name="hm")
            nc.sync.dma_start(out=hm[:], in_=hm_d)

            t3i = ep.tile([128, 3, 322], f32, name="t3i")  # unfold rows of inv-depth
            t3c = ep.tile([128, 3, 322], f32, name="t3c")  # unfold rows of conf
            # conf unfold rows straight from the host-padded input
            for nn in range(NN):
                src = dram_ap(
                    confpad_d, nn * 66 * 322,
                    [[322, 64], [322, 3], [1, 322]],
                )
                nc.sync.dma_start(out=t3c[nn * 64 : nn * 64 + 64], in_=src)

            inv_res = ep.tile([128, 330], f32, name="inv_res")
            zero2 = ep.tile([2, 132], f32, name="zero2")
            nc.vector.memset(zero2[:], 0.0)

            # ---------------- geometry ----------------
            u = pix[:, 0, :]
            v = pix[:, 1, :]
            d = pix[:, 2, :]
            fl = pix[:, 3, :]

            with tc.tile_pool(name="geom", bufs=1) as gp:
                _tagn = [0]

                def T(shape=(128, 330)):
                    _tagn[0] += 1
                    return gp.tile(list(shape), f32, name=f"g{_tagn[0]}", tag=f"g{_tagn[0]}")

                def TT(o, a, b, alu):
                    nc.vector.tensor_tensor(out=o, in0=a, in1=b, op=alu)

                def TS(o, a, s1, o0, s2=None, o1=None):
                    if o1 is None:
                        nc.vector.tensor_scalar(out=o, in0=a, scalar1=s1, scalar2=None, op0=o0)
                    else:
                        nc.vector.tensor_scalar(out=o, in0=a, scalar1=s1, scalar2=s2, op0=o0, op1=o1)

                def STT(o, a, s, b, o0, o1):
                    nc.vector.scalar_tensor_tensor(out=o, in0=a, scalar=s, in1=b, op0=o0, op1=o1)

                def AB(o, a):
                    nc.scalar.activation(out=o, in_=a, func=Act.Abs)

                def AF(o, a, scale, bias):
                    nc.scalar.activation(out=o, in_=a, func=Act.Identity, scale=scale, bias=bias)

                def recip_acc(o, x):
                    t = T()
                    nc.vector.reciprocal_approx_accurate(out=o, in_=x, scratch=t[:])

                # a_j = M @ [u, v, 1]
                a0, a1, a2 = T(), T(), T()
                tmp = T()
                AF(tmp[:], u, CC(C_M00), CC(C_M02))
                STT(a0[:], v, CC(C_M01), tmp[:], op.mult, op.add)
                AF(tmp[:], u, CC(C_M10), CC(C_M12))
                STT(a1[:], v, CC(C_M11), tmp[:], op.mult, op.add)
                AF(tmp[:], u, CC(C_M20), CC(C_M22))
                STT(a2[:], v, CC(C_M21), tmp[:], op.mult, op.add)

                d10 = T()
                AF(d10[:], d, 1.0, CC(C_TEN))

                # z components and their reciprocals
                ps2, pe2, rs2, re2 = T(), T(), T(), T()
                m = T()
                TT(m[:], a2[:], d, op.mult)
                AF(ps2[:], m[:], 1.0, CC(C_T2))
                TT(m[:], a2[:], d10[:], op.mult)
                TT(pe2[:], m[:], ps2[:], op.add)
                AB(m[:], ps2[:])
                TS(m[:], m[:], EPS, op.add)
                recip_acc(rs2[:], m[:])
                AB(m[:], pe2[:])
                TS(m[:], m[:], EPS, op.add)
                recip_acc(re2[:], m[:])

                # x/y components, start and end projections
                pxs, pys, pxe, pye = T(), T(), T(), T()
                for aj, tj, po_s, po_e in ((a0, C_T0, pxs, pxe), (a1, C_T1, pys, pye)):
                    psj, pej = T((128, 330)), T((128, 330))
                    TT(m[:], aj[:], d, op.mult)
                    AF(psj[:], m[:], 1.0, CC(tj))
                    TT(m[:], aj[:], d10[:], op.mult)
                    TT(pej[:], m[:], psj[:], op.add)
                    TT(po_s[:], psj[:], rs2[:], op.mult)
                    TT(po_e[:], pej[:], re2[:], op.mult)

                fdx, fdy = T(), T()
                TT(fdx[:], pxe[:], pxs[:], op.subtract)
                TT(fdy[:], pye[:], pys[:], op.subtract)

                # rsqrt(fdx^2 + fdy^2) via magic seed + 2 fused Newton steps
                q = T()
                nc.vector._custom_dve(cops["SUMSQ_ANT"], out=q[:], in0=fdx[:], in1=fdy[:])
                y = T()
                yi = y[:].bitcast(i32)
                TS(yi, q[:].bitcast(i32), 1, op.arith_shift_right)
                TS(yi, yi, -1, op.bitwise_xor)
                TS(yi, yi, 0x5F3759DF + 1, op.add)
                y2 = T()
                nc.vector._custom_dve(
                    cops["RSQRT_NR_ANT"], out=y2[:], in0=y[:], in1=q[:], s0=-0.5, s1=1.5
                )
                nc.vector._custom_dve(
                    cops["RSQRT_NR_ANT"], out=y[:], in0=y2[:], in1=q[:], s0=-0.5, s1=1.5
                )

                fls = T()
                TT(fls[:], fl, y[:], op.mult)
                mx, my = T(), T()
                TT(m[:], fdx[:], fls[:], op.mult)
                TT(mx[:], m[:], pxs[:], op.add)
                TT(m[:], fdy[:], fls[:], op.mult)
                TT(my[:], m[:], pys[:], op.add)

                fm = T()
                fmi = fm[:].bitcast(i32)
                ax = T()
                AB(ax[:], fdx[:])
                AB(m[:], fdy[:])
                TT(fmi, m[:], ax[:], op.is_gt)

                nx, ny = T(), T()
                AF(tmp[:], mx[:], CC(C_A0), CC(C_A2))
                STT(nx[:], my[:], CC(C_A1), tmp[:], op.mult, op.add)
                AF(tmp[:], mx[:], CC(C_B0), CC(C_B2))
                STT(ny[:], my[:], CC(C_B1), tmp[:], op.mult, op.add)

                rx, ry, rz = T(), T(), T()
                AF(tmp[:], u, CC(C_R00), CC(C_R02))
                STT(rx[:], v, CC(C_R01), tmp[:], op.mult, op.add)
                AF(tmp[:], u, CC(C_R10), CC(C_R12))
                STT(ry[:], v, CC(C_R11), tmp[:], op.mult, op.add)
                AF(tmp[:], u, CC(C_R20), CC(C_R22))
                STT(rz[:], v, CC(C_R21), tmp[:], op.mult, op.add)

                def inv_axis(o, nj, rj, c_t):
                    num = T()
                    TT(m[:], rz[:], nj[:], op.mult)
                    TT(m[:], rj[:], m[:], op.subtract)
                    AB(num[:], m[:])
                    AF(m[:], nj[:], CC(C_TZ), CC(c_t))
                    AB(m[:], m[:])
                    TS(m[:], m[:], EPS, op.add)
                    rden = T()
                    recip_acc(rden[:], m[:])
                    TT(o, num[:], rden[:], op.mult)

                invx, invy = T(), T()
                inv_axis(invx[:], nx, rx, C_TX)
                inv_axis(invy[:], ny, ry, C_TY)

                seld = T()
                nc.vector.select(out=seld[:], mask=fmi, on_true=invy[:], on_false=invx[:])
                AF(tmp[:], seld[:], CC(C_CA), CC(C_CB))
                TS(inv_res[:], tmp[:], 0.0, op.max, 1.0, op.min)

            # zero the halo pixels that fall outside the image (edge chunks)
            nc.vector.tensor_tensor(
                out=inv_res[:, 320:330], in0=inv_res[:, 320:330], in1=hm[:], op=op.mult
            )

            # ------- scratch round-trip: [nn, 66, 322] padded inv-depth -------
            for nn in range(NN):
                base = nn * 66 * 322
                sl = slice(nn * 64, nn * 64 + 64)
                nc.sync.dma_start(
                    out=dram_ap(scr, base + 322 + 1, [[322, 64], [1, 320]]),
                    in_=inv_res[sl, 0:320],
                )
                nc.sync.dma_start(
                    out=dram_ap(scr, base + 1, [[5, 64], [1, 5]]),
                    in_=inv_res[sl, 320:325],
                )
                nc.sync.dma_start(
                    out=dram_ap(scr, base + 65 * 322 + 1, [[5, 64], [1, 5]]),
                    in_=inv_res[sl, 325:330],
                )
                # zero pad columns 0 and 321 of all 66 rows
                nc.sync.dma_start(
                    out=dram_ap(scr, base, [[0, 1], [322, 66], [321, 2]]),
                    in_=zero2[nn : nn + 1, :].rearrange("p (a b) -> p a b", a=66),
                )
            for nn in range(NN):
                src = dram_ap(scr, nn * 66 * 322, [[322, 64], [322, 3], [1, 322]])
                nc.sync.dma_start(out=t3i[nn * 64 : nn * 64 + 64], in_=src)

            # 9 pre-shifted unfold rows per stream (one ACT copy per (dy,dx));
            # then per w-half expanded into exact scan-stream order so the
            # weighted-cumsum reads both ports unit-stride
            uf9i = pp.tile([128, 9, 324], f32, name="uf9i")
            uf9c = pp.tile([128, 9, 324], f32, name="uf9c")
            for t3, uf9 in ((t3i, uf9i), (t3c, uf9c)):
                for dy in range(3):
                    for dx in range(3):
                        nc.scalar.activation(
                            out=uf9[:, dy * 3 + dx, 0 : 322 - dx],
                            in_=t3[:, dy, dx:322],
                            func=Act.Copy,
                        )

            ep_ctx.__exit__(None, None, None)

            # ---------------- upsample + fusion, 2 w-halves x 4 p-chunks ----------------
            WC = 160
            with tc.tile_pool(name="chunk", bufs=2) as cp, tc.tile_pool(
                name="chunk1", bufs=1
            ) as cp1:
                for wc in range(2):
                    w0 = wc * WC
                    # [g(bcast), w, k] expansion of the unfold rows, unit stride
                    ufs = {}
                    for tag, uf9 in (("i", uf9i), ("c", uf9c)):
                        ust = cp1.tile([128, 4, WC, 9], f32, name="ufs" + tag, tag="ufs" + tag)
                        u9b = uf9[:]
                        pdim = list(u9b.ap[0])
                        for g in range(4):
                            nc.scalar.activation(
                                out=ust[:, g],
                                in_=bass.AP(
                                    tensor=u9b.tensor,
                                    offset=u9b.offset + w0,
                                    ap=[pdim, [1, WC], [324, 9]],
                                ),
                                func=Act.Copy,
                            )
                        ufs[tag] = ust
                    for gc in range(4):
                        e = cp.tile([128, 4, WC, 9], f32, name="e", tag="e")
                        nc.sync.dma_start(out=e[:], in_=mask_d[gc, wc])
                        nc.scalar.activation(out=e[:], in_=e[:], func=Act.Exp)

                        # softmax denominator: unit-stride innermost-k reduce
                        s = cp.tile([128, 4, WC], f32, name="s", tag="s")
                        nc.vector.tensor_reduce(
                            out=s[:], in_=e[:], axis=mybir.AxisListType.X, op=op.add
                        )
                        rs = cp.tile([128, 4, WC], f32, name="rs", tag="rs")
                        nc.vector.reciprocal_approx_fast(out=rs[:], in_=s[:])

                        up_t = {}
                        for tag in ("i", "c"):
                            cum = cp1.tile(
                                [128, 4, WC, 9], f32, name="cum", tag="cum"
                            )
                            for g in range(4):
                                nc.vector._custom_dve(
                                    cops["MUL_CUMSUM_ANT"], out=cum[:, g], in0=e[:, g], in1=ufs[tag][:, g]
                                )
                            # every-9th prefix value, with a zero column prepended
                            ce = cp1.tile([128, 4, WC + 1], f32, name="ce", tag="ce" + tag)
                            nc.vector.memset(ce[:, :, 0:1], 0.0)
                            nc.scalar.activation(
                                out=ce[:, :, 1 : WC + 1], in_=cum[:, :, :, 8], func=Act.Copy
                            )
                            acc = cp.tile([128, 4, WC], f32, name="acc", tag="acc" + tag)
                            nc.vector.tensor_tensor(
                                out=acc[:],
                                in0=ce[:, :, 1 : WC + 1],
                                in1=ce[:, :, 0:WC],
                                op=op.subtract,
                            )
                            upv = cp.tile([128, 4, WC], f32, name="upv", tag="up" + tag)
                            nc.vector.tensor_tensor(out=upv[:], in0=acc[:], in1=rs[:], op=op.mult)
                            up_t[tag] = upv

                        iu, cu = up_t["i"], up_t["c"]
                        lo, hi = slice(0, 64), slice(64, 128)

                        def F(tag):
                            return cp.tile([64, 4, WC], f32, name="f" + tag, tag="f" + tag)

                        # TT operands must share a base partition: move the nn1
                        # halves down to partitions 0-63 via SBUF->SBUF DMA
                        iu2, cu2 = F("iu2"), F("cu2")
                        nc.sync.dma_start(out=iu2[:], in_=iu[hi])
                        nc.sync.dma_start(out=cu2[:], in_=cu[hi])

                        fa, fb, fc, fd = F("a"), F("b"), F("c"), F("d")
                        # fa=dif -> fb=exp(dif) -> fc=1+fb -> fd=1/fc
                        nc.vector.tensor_tensor(out=fa[:], in0=cu2[:], in1=cu[lo], op=op.subtract)
                        nc.scalar.activation(out=fb[:], in_=fa[:], func=Act.Exp)
                        nc.scalar.activation(out=fc[:], in_=fb[:], func=Act.Identity, bias=1.0)
                        nc.vector.reciprocal_approx_fast(out=fd[:], in_=fc[:])
                        # fa=iu1*e -> fc=fa+iu0 -> fa=fc*fd -> fc=scale*fa+bias
                        nc.vector.tensor_tensor(out=fa[:], in0=iu2[:], in1=fb[:], op=op.mult)
                        nc.vector.tensor_tensor(out=fc[:], in0=fa[:], in1=iu[lo], op=op.add)
                        nc.vector.tensor_tensor(out=fa[:], in0=fc[:], in1=fd[:], op=op.mult)
                        nc.scalar.activation(
                            out=fc[:], in_=fa[:], func=Act.Identity,
                            scale=CC(C_DS, 0, 64), bias=CC(C_DB, 0, 64),
                        )
                        out_t = cp.tile([64, WC, 4], f32, name="out_t", tag="out_t")
                        nc.vector.reciprocal_approx_fast(
                            out=out_t[:].rearrange("p w q -> p q w"), in_=fc[:]
                        )
                        dst = dram_ap(
                            out_d,
                            gc * (W * UP) + UP * w0,
                            [[UP * W * UP, 64], [UP, WC], [1, UP]],
                        )
                        nc.sync.dma_start(out=dst, in_=out_t[:])

    nc.finalize()
    return nc


def _host_prep(inputs):
    K_ref = np.asarray(inputs["K_ref"], np.float32)
    K_nei = np.asarray(inputs["K_nei"], np.float32)
    R_nei = np.asarray(inputs["R_nei"], np.float32)
    T_nei = np.asarray(inputs["T_nei"], np.float32)
    depth0 = np.asarray(inputs["depth0"], np.float32)
    flow = np.asarray(inputs["flow"], np.float32)
    mask = np.asarray(inputs["mask"], np.float32)
    conf = np.asarray(inputs["conf"], np.float32)
    dmin = float(np.asarray(inputs["depth_min"]).reshape(-1)[0])
    dmax = float(np.asarray(inputs["depth_max"]).reshape(-1)[0])

    # pixel rays per batch (u, v with unit z)
    uv = []
    for b in range(B):
        Ki = np.linalg.inv(K_ref[b, 0, 0].astype(np.float64))
        gx, gy = np.meshgrid(np.arange(W, dtype=np.float64), np.arange(H, dtype=np.float64))
        x = Ki[0, 0] * gx + Ki[0, 1] * gy + Ki[0, 2]
        y = Ki[1, 0] * gx + Ki[1, 1] * gy + Ki[1, 2]
        z = Ki[2, 0] * gx + Ki[2, 1] * gy + Ki[2, 2]
        uv.append((np.float32(x / z), np.float32(y / z)))

    cA = 1.0 / (dmin - dmax)
    cB = -dmax / (dmin - dmax)

    in_maps = []
    for c in range(NCORES):
        b, rc = c // 4, c % 4
        r0 = rc * RPC
        rtop = max(r0 - 1, 0)
        rbot = min(r0 + RPC, H - 1)

        consts = np.zeros((128, NCONST), np.float32)
        for nn in range(NN):
            Kn = K_nei[nn, b, 0, 0].astype(np.float64)
            Rn = R_nei[nn, b, 0, 0].astype(np.float64)
            Tn = T_nei[nn, b, 0, 0].astype(np.float64).reshape(3)
            M = Kn @ Rn
            t = (Kn @ Tn.reshape(3, 1)).reshape(3)
            iK = np.linalg.inv(Kn)
            assert abs(iK[2, 0]) < 1e-12 and abs(iK[2, 1]) < 1e-12 and abs(iK[2, 2] - 1) < 1e-9
            row = np.zeros(NCONST, np.float32)
            row[C_M00:C_M22 + 1] = M.reshape(-1)
            row[C_T0:C_T2 + 1] = t
            row[C_R00:C_R22 + 1] = Rn.reshape(-1)
            row[C_A0:C_A2 + 1] = iK[0] / (1.0 + EPS)
            row[C_B0:C_B2 + 1] = iK[1] / (1.0 + EPS)
            # C_TX/C_TY feed |tz*n + c| as ACT affine bias -> store negated
            row[C_TX], row[C_TY], row[C_TZ] = -Tn[0], -Tn[1], Tn[2]
            row[C_CA], row[C_CB] = cA, cB
            row[C_TEN] = 10.0
            row[C_DS], row[C_DB] = dmin - dmax, dmax
            consts[nn * 64 : nn * 64 + 64] = row

        u_full, v_full = uv[b]
        d_full = depth0[b, 0]

        pix = np.zeros((128, 4, 330), np.float32)
        for nn in range(NN):
            sl = slice(nn * 64, nn * 64 + 64)
            f_full = flow[nn, b, 0]
            for ch, arr in enumerate((u_full, v_full, d_full, f_full)):
                pix[sl, ch, 0:320] = arr[r0 : r0 + RPC]
                pix[sl, ch, 320:325] = arr[rtop].reshape(64, 5)
                pix[sl, ch, 325:330] = arr[rbot].reshape(64, 5)

        hm = np.ones((128, 10), np.float32)
        if r0 == 0:
            hm[:, 0:5] = 0.0
        if r0 + RPC == H:
            hm[:, 5:10] = 0.0

        confpad = np.zeros((NN, 66, 322), np.float32)
        confpad[:, 1:65, 1:321] = conf[:, b, 0, r0 : r0 + RPC, :]
        if r0 > 0:
            confpad[:, 0, 1:321] = conf[:, b, 0, r0 - 1, :]
        if r0 + RPC < H:
            confpad[:, 65, 1:321] = conf[:, b, 0, r0 + RPC, :]

        # [nn, k, p, q, r, wc, w] -> [p, wc, nn, r, q, w, k]
        ms = mask[:, b, :, r0 : r0 + RPC, :].reshape(NN, 9, 4, 4, RPC, 2, 160)
        mask_pk = np.ascontiguousarray(ms.transpose(2, 5, 0, 4, 3, 6, 1)).reshape(
            4, 2, 128, 4, 160, 9
        )

        in_maps.append(
            {
                "pix": pix,
                "consts": consts,
                "hm": hm,
                "confpad": confpad,
                "maskpk": mask_pk,
            }
        )
    return in_maps


def kernel(**inputs):
    if "nc" not in _cache:
        _cache["nc"] = _build_program()
    nc = _cache["nc"]
    in_maps = _host_prep(inputs)

    from concourse import bass_utils

    res = bass_utils.run_bass_kernel_spmd(nc, in_maps, core_ids=list(range(NCORES)))
    out = np.empty((B, 1, H * UP, W * UP), np.float32)
    for c in range(NCORES):
        b, rc = c // 4, c % 4
        out[b, 0, rc * RPC * UP : (rc + 1) * RPC * UP, :] = res.results[c]["out"]
    return out


# revision 38
# speedup vs baseline: 1.1964x; 1.0528x over previous
"""DispMVS depth-fusion kernel for 8 Trainium2 NeuronCores.

Sharding: core c handles batch b = c // 4 and coarse rows r0 = (c % 4) * 64
(64 of 256 rows), with BOTH neighbor streams (NN=2) resident on the core
(partitions = nn*64 + row).  The cross-neighbor confidence-fusion softmax is
then core-local; cores never communicate.

Pipeline per core (one Bass/Tile program, identical for all 8 cores):
  1. geometry: elementwise epipolar math -> clipped inverse depth  [128, 330]
     (330 = 320 cols + 10 packed halo pixels/partition for the rows just
     outside the chunk, needed by the 3x3 unfold).
  2. DRAM scratch round-trip to rebuild inv-depth as 3 vertically shifted,
     zero-padded rows per partition (the unfold operand); conf comes the same
     way directly from a host-padded input.
  3. per (p, w-half) chunk: exp(mask) on ACT, grouped 9-way reductions on DVE
     (softmax numerators/denominator), convex-upsample of inv-depth and conf,
     then the 2-view softmax fusion and final reciprocal.
"""

import numpy as np

NN, B, H, W = 2, 2, 256, 320
UP = 4
EPS = 1e-6
RPC = 64          # coarse rows per core
NCORES = 8
HW = H * W
RW = RPC * W      # elements in one [64, 320] channel-slice

# consts columns
(
    C_M00, C_M01, C_M02, C_M10, C_M11, C_M12, C_M20, C_M21, C_M22,
    C_T0, C_T1, C_T2,
    C_R00, C_R01, C_R02, C_R10, C_R11, C_R12, C_R20, C_R21, C_R22,
    C_A0, C_A1, C_A2, C_B0, C_B1, C_B2,
    C_TX, C_TY, C_TZ,
    C_CA, C_CB, C_DS, C_DB, C_TEN,
) = range(35)
NCONST = 36

_cache = {}


def _register_custom_ops():
    """Register this kernel's custom DVE ops (idempotent). Returns a dict.

    MUL_CUMSUM_ANT: out = cumsum(in0*in1) along the free stream - grouped
      9-tap weighted sums fall out as differences of every-9th prefix value,
      one line-rate pass instead of multiply + strided TENSOR_REDUCE passes.
    SUMSQ_ANT: out = in0^2 + in1^2 (one pass instead of 3).
    RSQRT_NR_ANT: one Newton step for 1/sqrt: out = in0*(1.5 - 0.5*in1*in0^2)
      (one pass instead of 4).
    """
    from concourse import dve_ops
    from concourse.dve_spec import AluOp, C0, C1, Spec, Src0, Src1, _has_src1, lower, scan
    from concourse.dve_uop import DveOpSpec

    have = {o.name: o for o in dve_ops.OPS}
    if "MUL_CUMSUM_ANT" in have:
        return have

    def cum_ref(in0, in1, s0, s1, imm2):
        a = in0.astype(np.float32).reshape(in0.shape[0], -1) * in1.astype(
            np.float32
        ).reshape(in1.shape[0], -1)
        return np.cumsum(a, axis=1, dtype=np.float32).reshape(in0.shape)

    specs = [
        ("MUL_CUMSUM_ANT", Spec(body=scan(AluOp.ADD, Src0 * Src1), reference=cum_ref)),
        (
            "SUMSQ_ANT",
            Spec(
                body=Src0 * Src0 + Src1 * Src1,
                reference=lambda in0, in1, s0, s1, imm2: (
                    in0.astype(np.float32) ** 2 + in1.astype(np.float32) ** 2
                ),
            ),
        ),
        (
            "RSQRT_NR_ANT",
            Spec(
                body=(Src0 * Src0 * Src1 * C0 + C1) * Src0,
                reference=lambda in0, in1, s0, s1, imm2: (
                    (in0.astype(np.float32) ** 2 * in1 * s0 + s1) * in0
                ),
            ),
        ),
    ]
    out = {}
    for name, spec in specs:
        op = dve_ops.DveOp(name, spec, subdim=False, uops_sha={})
        dve_ops.OPS.append(op)
        dve_ops.CUSTOM_DVE_SPECS[name] = spec
        dve_ops._SUB_OPCODE_FOR_NAME[name] = (
            dve_ops._CUSTOM_DVE_ROW_BASE + len(dve_ops.OPS) - 1
        )
        for ver in ("v3", "v4"):
            tmp = DveOpSpec(
                name=name,
                opcode=dve_ops.get_dve_sub_opcode(name),
                uops=lower(spec, ver=ver),
                rd1_en=_has_src1(spec),
            )
            op.uops_sha[ver] = tmp.sha(ver)
        out[name] = op
    assert max(dve_ops._SUB_OPCODE_FOR_NAME.values()) < 0x20
    return out


def _build_program():
    import concourse.bass as bass
    import concourse.bacc as bacc
    import concourse.tile as tile
    from concourse import mybir
    from concourse.alu_op_type import AluOpType as op

    f32 = mybir.dt.float32
    i32 = mybir.dt.int32
    Act = mybir.ActivationFunctionType

    cops = _register_custom_ops()
    nc = bacc.Bacc("TRN2", target_bir_lowering=False, debug=False)

    pix_d = nc.dram_tensor("pix", [128, 4, 330], f32, kind="ExternalInput").ap()
    consts_d = nc.dram_tensor("consts", [128, NCONST], f32, kind="ExternalInput").ap()
    hm_d = nc.dram_tensor("hm", [128, 10], f32, kind="ExternalInput").ap()
    confpad_d = nc.dram_tensor("confpad", [NN, 66, 322], f32, kind="ExternalInput").ap()
    # mask pre-packed on host to [gc, wc, (nn,r), q, w, k] (k innermost) so each
    # chunk's DMA is one contiguous [128, 5760] transfer and the 9-tap groups
    # are unit-stride for the cumsum trick
    mask_d = nc.dram_tensor("maskpk", [4, 2, 128, 4, 160, 9], f32, kind="ExternalInput").ap()
    scr = nc.dram_tensor("scr", [NN, 66, 322], f32, kind="Internal").ap()
    out_d = nc.dram_tensor("out", [RPC * UP, W * UP], f32, kind="ExternalOutput").ap()

    def dram_ap(base, off, dims):
        return bass.AP(tensor=base.tensor, offset=base.offset + off, ap=[list(d) for d in dims])

    with tile.TileContext(nc) as tc:
        with tc.tile_pool(name="persist", bufs=1) as pp:
            ep_ctx = tc.tile_pool(name="early", bufs=1)
            ep = ep_ctx.__enter__()
            consts = pp.tile([128, NCONST], f32, name="consts")
            nc.sync.dma_start(out=consts[:], in_=consts_d)

            def CC(i, p0=0, p1=128):
                return consts[p0:p1, i : i + 1]

            pix = ep.tile([128, 4, 330], f32, name="pix")
            nc.sync.dma_start(out=pix[:], in_=pix_d)
            hm = ep.tile([128, 10], f32, name="hm")
            nc.sync.dma_start(out=hm[:], in_=hm_d)

            t3i = ep.tile([128, 3, 322], f32, name="t3i")  # unfold rows of inv-depth
            t3c = ep.tile([128, 3, 322], f32, name="t3c")  # unfold rows of conf
            # conf unfold rows straight from the host-padded input
            for nn in range(NN):
                src = dram_ap(
                    confpad_d, nn * 66 * 322,
                    [[322, 64], [322, 3], [1, 322]],
                )
                nc.sync.dma_start(out=t3c[nn * 64 : nn * 64 + 64], in_=src)

            inv_res = ep.tile([128, 330], f32, name="inv_res")
            zero2 = ep.tile([2, 132], f32, name="zero2")
            nc.vector.memset(zero2[:], 0.0)

            # ---------------- geometry ----------------
            u = pix[:, 0, :]
            v = pix[:, 1, :]
            d = pix[:, 2, :]
            fl = pix[:, 3, :]

            with tc.tile_pool(name="geom", bufs=1) as gp:
                _tagn = [0]

                def T(shape=(128, 330)):
                    _tagn[0] += 1
                    return gp.tile(list(shape), f32, name=f"g{_tagn[0]}", tag=f"g{_tagn[0]}")

                def TT(o, a, b, alu):
                    nc.vector.tensor_tensor(out=o, in0=a, in1=b, op=alu)

                def TS(o, a, s1, o0, s2=None, o1=None):
                    if o1 is None:
                        nc.vector.tensor_scalar(out=o, in0=a, scalar1=s1, scalar2=None, op0=o0)
                    else:
                        nc.vector.tensor_scalar(out=o, in0=a, scalar1=s1, scalar2=s2, op0=o0, op1=o1)

                def STT(o, a, s, b, o0, o1):
                    nc.vector.scalar_tensor_tensor(out=o, in0=a, scalar=s, in1=b, op0=o0, op1=o1)

                def AB(o, a):
                    nc.scalar.activation(out=o, in_=a, func=Act.Abs)

                def AF(o, a, scale, bias):
                    nc.scalar.activation(out=o, in_=a, func=Act.Identity, scale=scale, bias=bias)

                def recip_acc(o, x):
                    t = T()
                    nc.vector.reciprocal_approx_accurate(out=o, in_=x, scratch=t[:])

                # a_j = M @ [u, v, 1]
                a0, a1, a2 = T(), T(), T()
                tmp = T()
                AF(tmp[:], u, CC(C_M00), CC(C_M02))
                STT(a0[:], v, CC(C_M01), tmp[:], op.mult, op.add)
                AF(tmp[:], u, CC(C_M10), CC(C_M12))
                STT(a1[:], v, CC(C_M11), tmp[:], op.mult, op.add)
                AF(tmp[:], u, CC(C_M20), CC(C_M22))
                STT(a2[:], v, CC(C_M21), tmp[:], op.mult, op.add)

                d10 = T()
                AF(d10[:], d, 1.0, CC(C_TEN))

                # z components and their reciprocals
                ps2, pe2, rs2, re2 = T(), T(), T(), T()
                m = T()
                TT(m[:], a2[:], d, op.mult)
                AF(ps2[:], m[:], 1.0, CC(C_T2))
                TT(m[:], a2[:], d10[:], op.mult)
                TT(pe2[:], m[:], ps2[:], op.add)
                AB(m[:], ps2[:])
                TS(m[:], m[:], EPS, op.add)
                recip_acc(rs2[:], m[:])
                AB(m[:], pe2[:])
                TS(m[:], m[:], EPS, op.add)
                recip_acc(re2[:], m[:])

                # x/y components, start and end projections
                pxs, pys, pxe, pye = T(), T(), T(), T()
                for aj, tj, po_s, po_e in ((a0, C_T0, pxs, pxe), (a1, C_T1, pys, pye)):
                    psj, pej = T((128, 330)), T((128, 330))
                    TT(m[:], aj[:], d, op.mult)
                    AF(psj[:], m[:], 1.0, CC(tj))
                    TT(m[:], aj[:], d10[:], op.mult)
                    TT(pej[:], m[:], psj[:], op.add)
                    TT(po_s[:], psj[:], rs2[:], op.mult)
                    TT(po_e[:], pej[:], re2[:], op.mult)

                fdx, fdy = T(), T()
                TT(fdx[:], pxe[:], pxs[:], op.subtract)
                TT(fdy[:], pye[:], pys[:], op.subtract)

                # rsqrt(fdx^2 + fdy^2) via magic seed + 2 fused Newton steps
                q = T()
                nc.vector._custom_dve(cops["SUMSQ_ANT"], out=q[:], in0=fdx[:], in1=fdy[:])
                y = T()
                yi = y[:].bitcast(i32)
                TS(yi, q[:].bitcast(i32), 1, op.arith_shift_right)
                TS(yi, yi, -1, op.bitwise_xor)
                TS(yi, yi, 0x5F3759DF + 1, op.add)
                y2 = T()
                nc.vector._custom_dve(
                    cops["RSQRT_NR_ANT"], out=y2[:], in0=y[:], in1=q[:], s0=-0.5, s1=1.5
                )
                nc.vector._custom_dve(
                    cops["RSQRT_NR_ANT"], out=y[:], in0=y2[:], in1=q[:], s0=-0.5, s1=1.5
                )

                fls = T()
                TT(fls[:], fl, y[:], op.mult)
                mx, my = T(), T()
                TT(m[:], fdx[:], fls[:], op.mult)
                TT(mx[:], m[:], pxs[:], op.add)
                TT(m[:], fdy[:], fls[:], op.mult)
                TT(my[:], m[:], pys[:], op.add)

                fm = T()
                fmi = fm[:].bitcast(i32)
                ax = T()
                AB(ax[:], fdx[:])
                AB(m[:], fdy[:])
                TT(fmi, m[:], ax[:], op.is_gt)

                nx, ny = T(), T()
                AF(tmp[:], mx[:], CC(C_A0), CC(C_A2))
                STT(nx[:], my[:], CC(C_A1), tmp[:], op.mult, op.add)
                AF(tmp[:], mx[:], CC(C_B0), CC(C_B2))
                STT(ny[:], my[:], CC(C_B1), tmp[:], op.mult, op.add)

                rx, ry, rz = T(), T(), T()
                AF(tmp[:], u, CC(C_R00), CC(C_R02))
                STT(rx[:], v, CC(C_R01), tmp[:], op.mult, op.add)
                AF(tmp[:], u, CC(C_R10), CC(C_R12))
                STT(ry[:], v, CC(C_R11), tmp[:], op.mult, op.add)
                AF(tmp[:], u, CC(C_R20), CC(C_R22))
                STT(rz[:], v, CC(C_R21), tmp[:], op.mult, op.add)

                def inv_axis(o, nj, rj, c_t):
                    num = T()
                    TT(m[:], rz[:], nj[:], op.mult)
                    TT(m[:], rj[:], m[:], op.subtract)
                    AB(num[:], m[:])
                    AF(m[:], nj[:], CC(C_TZ), CC(c_t))
                    AB(m[:], m[:])
                    TS(m[:], m[:], EPS, op.add)
                    rden = T()
                    recip_acc(rden[:], m[:])
                    TT(o, num[:], rden[:], op.mult)

                invx, invy = T(), T()
                inv_axis(invx[:], nx, rx, C_TX)
                inv_axis(invy[:], ny, ry, C_TY)

                seld = T()
                nc.vector.select(out=seld[:], mask=fmi, on_true=invy[:], on_false=invx[:])
                AF(tmp[:], seld[:], CC(C_CA), CC(C_CB))
                TS(inv_res[:], tmp[:], 0.0, op.max, 1.0, op.min)

            # zero the halo pixels that fall outside the image (edge chunks)
            nc.vector.tensor_tensor(
                out=inv_res[:, 320:330], in0=inv_res[:, 320:330], in1=hm[:], op=op.mult
            )

            # ------- scratch round-trip: [nn, 66, 322] padded inv-depth -------
            for nn in range(NN):
                base = nn * 66 * 322
                sl = slice(nn * 64, nn * 64 + 64)
                nc.sync.dma_start(
                    out=dram_ap(scr, base + 322 + 1, [[322, 64], [1, 320]]),
                    in_=inv_res[sl, 0:320],
                )
                nc.sync.dma_start(
                    out=dram_ap(scr, base + 1, [[5, 64], [1, 5]]),
                    in_=inv_res[sl, 320:325],
                )
                nc.sync.dma_start(
                    out=dram_ap(scr, base + 65 * 322 + 1, [[5, 64], [1, 5]]),
                    in_=inv_res[sl, 325:330],
                )
                # zero pad columns 0 and 321 of all 66 rows
                nc.sync.dma_start(
                    out=dram_ap(scr, base, [[0, 1], [322, 66], [321, 2]]),
                    in_=zero2[nn : nn + 1, :].rearrange("p (a b) -> p a b", a=66),
                )
            for nn in range(NN):
                src = dram_ap(scr, nn * 66 * 322, [[322, 64], [322, 3], [1, 322]])
                nc.sync.dma_start(out=t3i[nn * 64 : nn * 64 + 64], in_=src)

            # unfold weights interleaved [w, k] (k innermost) so the
            # weighted-cumsum's src1 for any w-window is one contiguous slice
            ufi9i = pp.tile([128, 322, 9], f32, name="ufi9i")
            ufi9c = pp.tile([128, 322, 9], f32, name="ufi9c")
            for t3, ufi9 in ((t3i, ufi9i), (t3c, ufi9c)):
                for dy in range(3):
                    for dx in range(3):
                        nc.scalar.activation(
                            out=ufi9[:, 0 : 322 - dx, dy * 3 + dx],
                            in_=t3[:, dy, dx:322],
                            func=Act.Copy,
                        )

            ep_ctx.__exit__(None, None, None)

            # ---------------- upsample + fusion, 2 w-halves x 4 p-chunks ----------------
            WC = 160
            with tc.tile_pool(name="chunk", bufs=2) as cp, tc.tile_pool(
                name="chunk1", bufs=1
            ) as cp1:
                for wc in range(2):
                    w0 = wc * WC
                    ufs = {"i": ufi9i[:, w0 : w0 + WC, :], "c": ufi9c[:, w0 : w0 + WC, :]}
                    for gc in range(4):
                        e = cp.tile([128, 4, WC, 9], f32, name="e", tag="e")
                        nc.sync.dma_start(out=e[:], in_=mask_d[gc, wc])
                        nc.scalar.activation(out=e[:], in_=e[:], func=Act.Exp)

                        # softmax denominator: unit-stride innermost-k reduce
                        s = cp.tile([128, 4, WC], f32, name="s", tag="s")
                        nc.vector.tensor_reduce(
                            out=s[:], in_=e[:], axis=mybir.AxisListType.X, op=op.add
                        )
                        rs = cp.tile([128, 4, WC], f32, name="rs", tag="rs")
                        nc.vector.reciprocal_approx_fast(out=rs[:], in_=s[:])

                        up_t = {}
                        for tag in ("i", "c"):
                            cum = cp1.tile(
                                [128, 4, WC, 9], f32, name="cum", tag="cum", bufs=2
                            )
                            for g in range(4):
                                nc.vector._custom_dve(
                                    cops["MUL_CUMSUM_ANT"], out=cum[:, g], in0=e[:, g], in1=ufs[tag]
                                )
                            # every-9th prefix value, with a zero column prepended
                            ce = cp1.tile([128, 4, WC + 1], f32, name="ce", tag="ce" + tag)
                            nc.vector.memset(ce[:, :, 0:1], 0.0)
                            nc.scalar.activation(
                                out=ce[:, :, 1 : WC + 1], in_=cum[:, :, :, 8], func=Act.Copy
                            )
                            acc = cp.tile([128, 4, WC], f32, name="acc", tag="acc" + tag)
                            nc.vector.tensor_tensor(
                                out=acc[:],
                                in0=ce[:, :, 1 : WC + 1],
                                in1=ce[:, :, 0:WC],
                                op=op.subtract,
                            )
                            upv = cp.tile([128, 4, WC], f32, name="upv", tag="up" + tag)
                            nc.vector.tensor_tensor(out=upv[:], in0=acc[:], in1=rs[:], op=op.mult)
                            up_t[tag] = upv

                        iu, cu = up_t["i"], up_t["c"]
                        lo, hi = slice(0, 64), slice(64, 128)

                        def F(tag):
                            return cp.tile([64, 4, WC], f32, name="f" + tag, tag="f" + tag)

                        # TT operands must share a base partition: move the nn1
                        # halves down to partitions 0-63 via SBUF->SBUF DMA
                        iu2, cu2 = F("iu2"), F("cu2")
                        nc.sync.dma_start(out=iu2[:], in_=iu[hi])
                        nc.sync.dma_start(out=cu2[:], in_=cu[hi])

                        fa, fb, fc, fd = F("a"), F("b"), F("c"), F("d")
                        # fa=dif -> fb=exp(dif) -> fc=1+fb -> fd=1/fc
                        nc.vector.tensor_tensor(out=fa[:], in0=cu2[:], in1=cu[lo], op=op.subtract)
                        nc.scalar.activation(out=fb[:], in_=fa[:], func=Act.Exp)
                        nc.scalar.activation(out=fc[:], in_=fb[:], func=Act.Identity, bias=1.0)
                        nc.vector.reciprocal_approx_fast(out=fd[:], in_=fc[:])
                        # fa=iu1*e -> fc=fa+iu0 -> fa=fc*fd -> fc=scale*fa+bias
                        nc.vector.tensor_tensor(out=fa[:], in0=iu2[:], in1=fb[:], op=op.mult)
                        nc.vector.tensor_tensor(out=fc[:], in0=fa[:], in1=iu[lo], op=op.add)
                        nc.vector.tensor_tensor(out=fa[:], in0=fc[:], in1=fd[:], op=op.mult)
                        nc.scalar.activation(
                            out=fc[:], in_=fa[:], func=Act.Identity,
                            scale=CC(C_DS, 0, 64), bias=CC(C_DB, 0, 64),
                        )
                        out_t = cp.tile([64, WC, 4], f32, name="out_t", tag="out_t")
                        nc.vector.reciprocal_approx_fast(
                            out=out_t[:].rearrange("p w q -> p q w"), in_=fc[:]
                        )
                        dst = dram_ap(
                            out_d,
                            gc * (W * UP) + UP * w0,
                            [[UP * W * UP, 64], [UP, WC], [1, UP]],
                        )
                        nc.sync.dma_start(out=dst, in_=out_t[:])

    nc.finalize()
    return nc


def _host_prep(inputs):
    K_ref = np.asarray(inputs["K_ref"], np.float32)
    K_nei = np.asarray(inputs["K_nei"], np.float32)
    R_nei = np.asarray(inputs["R_nei"], np.float32)
    T_nei = np.asarray(inputs["T_nei"], np.float32)
    depth0 = np.asarray(inputs["depth0"], np.float32)
    flow = np.asarray(inputs["flow"], np.float32)
    mask = np.asarray(inputs["mask"], np.float32)
    conf = np.asarray(inputs["conf"], np.float32)
    dmin = float(np.asarray(inputs["depth_min"]).reshape(-1)[0])
    dmax = float(np.asarray(inputs["depth_max"]).reshape(-1)[0])

    # pixel rays per batch (u, v with unit z)
    uv = []
    for b in range(B):
        Ki = np.linalg.inv(K_ref[b, 0, 0].astype(np.float64))
        gx, gy = np.meshgrid(np.arange(W, dtype=np.float64), np.arange(H, dtype=np.float64))
        x = Ki[0, 0] * gx + Ki[0, 1] * gy + Ki[0, 2]
        y = Ki[1, 0] * gx + Ki[1, 1] * gy + Ki[1, 2]
        z = Ki[2, 0] * gx + Ki[2, 1] * gy + Ki[2, 2]
        uv.append((np.float32(x / z), np.float32(y / z)))

    cA = 1.0 / (dmin - dmax)
    cB = -dmax / (dmin - dmax)

    in_maps = []
    for c in range(NCORES):
        b, rc = c // 4, c % 4
        r0 = rc * RPC
        rtop = max(r0 - 1, 0)
        rbot = min(r0 + RPC, H - 1)

        consts = np.zeros((128, NCONST), np.float32)
        for nn in range(NN):
            Kn = K_nei[nn, b, 0, 0].astype(np.float64)
            Rn = R_nei[nn, b, 0, 0].astype(np.float64)
            Tn = T_nei[nn, b, 0, 0].astype(np.float64).reshape(3)
            M = Kn @ Rn
            t = (Kn @ Tn.reshape(3, 1)).reshape(3)
            iK = np.linalg.inv(Kn)
            assert abs(iK[2, 0]) < 1e-12 and abs(iK[2, 1]) < 1e-12 and abs(iK[2, 2] - 1) < 1e-9
            row = np.zeros(NCONST, np.float32)
            row[C_M00:C_M22 + 1] = M.reshape(-1)
            row[C_T0:C_T2 + 1] = t
            row[C_R00:C_R22 + 1] = Rn.reshape(-1)
            row[C_A0:C_A2 + 1] = iK[0] / (1.0 + EPS)
            row[C_B0:C_B2 + 1] = iK[1] / (1.0 + EPS)
            # C_TX/C_TY feed |tz*n + c| as ACT affine bias -> store negated
            row[C_TX], row[C_TY], row[C_TZ] = -Tn[0], -Tn[1], Tn[2]
            row[C_CA], row[C_CB] = cA, cB
            row[C_TEN] = 10.0
            row[C_DS], row[C_DB] = dmin - dmax, dmax
            consts[nn * 64 : nn * 64 + 64] = row

        u_full, v_full = uv[b]
        d_full = depth0[b, 0]

        pix = np.zeros((128, 4, 330), np.float32)
        for nn in range(NN):
            sl = slice(nn * 64, nn * 64 + 64)
            f_full = flow[nn, b, 0]
            for ch, arr in enumerate((u_full, v_full, d_full, f_full)):
                pix[sl, ch, 0:320] = arr[r0 : r0 + RPC]
                pix[sl, ch, 320:325] = arr[rtop].reshape(64, 5)
                pix[sl, ch, 325:330] = arr[rbot].reshape(64, 5)

        hm = np.ones((128, 10), np.float32)
        if r0 == 0:
            hm[:, 0:5] = 0.0
        if r0 + RPC == H:
            hm[:, 5:10] = 0.0

        confpad = np.zeros((NN, 66, 322), np.float32)
        confpad[:, 1:65, 1:321] = conf[:, b, 0, r0 : r0 + RPC, :]
        if r0 > 0:
            confpad[:, 0, 1:321] = conf[:, b, 0, r0 - 1, :]
        if r0 + RPC < H:
            confpad[:, 65, 1:321] = conf[:, b, 0, r0 + RPC, :]

        # [nn, k, p, q, r, wc, w] -> [p, wc, nn, r, q, w, k]
        ms = mask[:, b, :, r0 : r0 + RPC, :].reshape(NN, 9, 4, 4, RPC, 2, 160)
        mask_pk = np.ascontiguousarray(ms.transpose(2, 5, 0, 4, 3, 6, 1)).reshape(
            4, 2, 128, 4, 160, 9
        )

        in_maps.append(
            {
                "pix": pix,
                "consts": consts,
                "hm": hm,
                "confpad": confpad,
                "maskpk": mask_pk,
            }
        )
    return in_maps


def kernel(**inputs):
    if "nc" not in _cache:
        _cache["nc"] = _build_program()
    nc = _cache["nc"]
    in_maps = _host_prep(inputs)

    from concourse import bass_utils

    res = bass_utils.run_bass_kernel_spmd(nc, in_maps, core_ids=list(range(NCORES)))
    out = np.empty((B, 1, H * UP, W * UP), np.float32)
    for c in range(NCORES):
        b, rc = c // 4, c % 4
        out[b, 0, rc * RPC * UP : (rc + 1) * RPC * UP, :] = res.results[c]["out"]
    return out
